# revision 25
# baseline (speedup 1.0000x reference)
"""PillarMaxPoolingV2a on 8 TRN2 NeuronCores (Bass/Tile) - v12.

Device architecture (per core = one batch, ~213k padded points):
  - feats bf16 channel-major [128, CW]: column c packs 4 points:
    rows 0:32   = channels of point (s=0, l=0), rows 32:64 = (s=0, l=1)
    rows 64:96  = (s=1, l=0), rows 96:128 = (s=1, l=1)
  - block-diagonal W [128, 128] bf16; stream s matmul: lhsT=wcm[64s:64s+64],
    rhs=f[64s:64s+64, cols] -> PSUM [128, cols]: rows 0:64 = 64 out-channels
    of the l=0 point of each column, rows 64:128 = l=1 point.  The two
    streams run on disjoint PE row-quadrants.
  - The kernel is a flat list of STEPS (1024 feats cols = 4096 points each;
    two [128,1024] fp32 PSUM duos from a 4-buf pool).  Step types:
    "AA": both duos window-8 tensor_reduce (DVE, from PSUM), members
          consecutive (col 8g+m).  2 x 128 out cols, fully reduced.
    "AB": s0 like AA; s1: ScalarE copy -> SBUF bf16, members self-
          interleaved (col 128m+g), one contiguous-halves TT max (2x mode)
          -> 4 partial cols per group (host folds).  128 + 512 out cols.
    "B2": 512 w8-groups pair-interleaved across both duos (member m of
          group (l,j): m<4 at s0-col 256m+j, m>=4 at s1-col 256(m-4)+j);
          two ScalarE copies into one [128,2048] collect, one TT max
          -> 4 partials per group.  1024 out cols.
    "W4": 1024 window-4 groups (pillar tails, r in 1..4, padded to 4):
          member m<2 at s0-col 512m+j, m>=2 at s1-col 512(m-2)+j; two
          copies + one TT -> 2 partials per group.  1024 out cols.
  - Tree TTs are deferred one step so the DVE queue never head-of-line
    blocks on a ScalarE copy.  feats DMA per 2 steps; output staged per
    4 steps, DMA'd out via GPSIMD SWDGE (separate queue from input).
Host: pillar sort; per pillar floor(c/8) full w8-groups, one padded
w8-group if r>=5, one padded w4-group if 1<=r<=4 (empty pillars get no
group).  Gather/scatter layouts above; final fold + per-pillar combine +
bias + ReLU + masking on host.
"""
import math
import numpy as np
import sys

sys.path.insert(0, "/opt/trn_rl_repo")

import ml_dtypes

BF16 = ml_dtypes.bfloat16

# ---- problem constants (hardcoded per contract) ----
B = 8
NPOINTS = 1_600_000
C_IN = 29
C_MLP0 = 32
C_OUT = 64
BEV = np.float32(0.8)
X_MIN = np.float32(0.0)
Y_MIN = np.float32(-40.0)
Z_MIN = np.float32(-3.0)
W = 88
H = 100
EPS = np.float32(1e-5)
NPIL = H * W            # 8800 pillars per batch
GK = 8

# w8-step mix pattern (cycled); W4 steps are interleaved separately
P8 = ("AA", "B2", "AB", "B2", "AA", "B2", "B2", "AA")
STEP_W = {"AA": 256, "AB": 640, "B2": 1024, "W4": 1024}

_prog_cache = {}
_debug_state = {}


def _make_steps(n8, n4):
    """Interleave n8 w8-steps (cycling P8) with n4 W4 steps, spread out."""
    steps = []
    i8 = 0
    if n4:
        gap = max(1, round(n8 / n4))
        for i4 in range(n4):
            take = min(gap, n8 - i8)
            for _ in range(take):
                steps.append(P8[i8 % len(P8)])
                i8 += 1
            steps.append("W4")
        while i8 < n8:
            steps.append(P8[i8 % len(P8)])
            i8 += 1
    else:
        steps = [P8[i % len(P8)] for i in range(n8)]
    # drain optimization: end with AA steps (V-direct, no collect->tree
    # chain), so the final stage flushes quickly
    tail = steps[-8:]
    steps[-8:] = [s for s in tail if s != "AA"] + [s for s in tail if s == "AA"]
    return tuple(steps)


def _step_offsets(steps):
    """Per-step output col base and total output width."""
    ob = []
    off = 0
    for st in steps:
        ob.append(off)
        off += STEP_W[st]
    return ob, off


def _build_program(steps):
    """Device program for the given step list."""
    import concourse.bass as bass
    import concourse.bacc as bacc
    import concourse.mybir as mybir
    import concourse.tile as tile
    import contextlib

    MAX = mybir.AluOpType.max
    X = mybir.AxisListType.X
    nst = len(steps)
    ob_of, wout = _step_offsets(steps)
    cw = nst * 1024

    nc = bacc.Bacc("TRN2", target_bir_lowering=False, debug=False, num_devices=8)
    feats_in = nc.declare_dram_parameter("feats", [128, cw], mybir.dt.bfloat16,
                                         isOutput=False)
    w_in = nc.declare_dram_parameter("wcm", [128, 128], mybir.dt.bfloat16,
                                     isOutput=False)
    out = nc.declare_dram_parameter("outr", [128, wout],
                                    mybir.dt.bfloat16, isOutput=True)

    with tile.TileContext(nc) as tc:
        with contextlib.ExitStack() as stack:
            constp = stack.enter_context(tc.tile_pool(name="const", bufs=1))
            featsp = stack.enter_context(tc.tile_pool(name="feats", bufs=6))
            psump = stack.enter_context(
                tc.tile_pool(name="psum", bufs=4, space="PSUM"))
            bcollp = stack.enter_context(tc.tile_pool(name="bcoll", bufs=4))
            b2collp = stack.enter_context(tc.tile_pool(name="b2coll", bufs=3))
            stgap = stack.enter_context(tc.tile_pool(name="stga", bufs=3))

            wcm = constp.tile([128, 128], mybir.dt.bfloat16)
            nc.sync.dma_start(out=wcm[:], in_=w_in[:])

            # PE warm-up: ~5.6us of back-to-back matmuls during the boot
            # window flips the HAM clock gate to K=8/8 before real work;
            # steady-state PE duty then keeps it warm.
            warm = psump.tile([128, 1024], mybir.dt.float32, tag="duo")
            for _ in range(26):
                nc.tensor.matmul(
                    out=warm[:, 0:128],
                    lhsT=wcm[0:64, :],
                    rhs=wcm[0:64, :],
                    start=True, stop=True,
                )

            def emit_tree(job):
                # one contiguous-halves TT max; host folds the partials
                bc, n, dst = job
                h = n // 2
                nc.vector.tensor_tensor(
                    out=dst, in0=bc[:, 0:h], in1=bc[:, h:n], op=MAX)

            fh = None
            stga = None
            stga_done = 0
            stga_base = 0
            pending = []
            for k, st in enumerate(steps):
                if k % 2 == 0:
                    nh = min(2, nst - k)
                    fh = featsp.tile([128, 2048], mybir.dt.bfloat16,
                                     tag="ftile")
                    nc.sync.dma_start(
                        out=fh[:, 0:1024 * nh],
                        in_=feats_in[:, 1024 * k:1024 * (k + nh)])
                if stga is None:
                    lastk = min(k + 4, nst)
                    pw = (ob_of[lastk] if lastk < nst else wout) - ob_of[k]
                    stga_base = ob_of[k]
                    stga = stgap.tile([128, pw], mybir.dt.bfloat16, tag="stga")
                    stga_done = lastk

                c0 = 1024 * (k % 2)
                ob = ob_of[k] - stga_base
                duos = []
                for s in range(2):
                    p = psump.tile([128, 1024], mybir.dt.float32, tag="duo")
                    duos.append(p)
                for jj in range(2):
                    for s in range(2):
                        nc.tensor.matmul(
                            out=duos[s][:, 512 * jj:512 * (jj + 1)],
                            lhsT=wcm[64 * s:64 * (s + 1), :],
                            rhs=fh[64 * s:64 * (s + 1),
                                   c0 + 512 * jj:c0 + 512 * (jj + 1)],
                            start=True, stop=True,
                        )
                if st == "AA" or st == "AB":
                    nc.vector.tensor_reduce(
                        out=stga[:, ob:ob + 128],
                        in_=duos[0][:].rearrange("p (g k) -> p g k", k=GK),
                        axis=X, op=MAX,
                    )
                    if st == "AA":
                        nc.vector.tensor_reduce(
                            out=stga[:, ob + 128:ob + 256],
                            in_=duos[1][:].rearrange("p (g k) -> p g k", k=GK),
                            axis=X, op=MAX,
                        )
                    else:
                        bc = bcollp.tile([128, 1024], mybir.dt.bfloat16,
                                         tag="bcl")
                        nc.scalar.copy(out=bc[:], in_=duos[1][:])
                        pending.append((bc, 1024, stga[:, ob + 128:ob + 640]))
                else:  # B2 / W4
                    bc = b2collp.tile([128, 2048], mybir.dt.bfloat16,
                                      tag="b2cl")
                    nc.scalar.copy(out=bc[:, 0:1024], in_=duos[0][:])
                    nc.scalar.copy(out=bc[:, 1024:2048], in_=duos[1][:])
                    pending.append((bc, 2048, stga[:, ob:ob + 1024]))
                while len(pending) > 1:
                    emit_tree(pending.pop(0))

                if k + 1 == stga_done:
                    while pending:
                        emit_tree(pending.pop(0))
                    nc.gpsimd.dma_start(
                        out=out[:, stga_base:stga_base + stga.shape[1]],
                        in_=stga[:],
                    )
                    stga = None
    nc.compile()
    return nc


def _group_layout(pid, counts):
    """Two-class grouping.  Returns (src8 [G8,8], pil8 [G8],
    src4 [G4,4], pil4 [G4]) with per-pillar order: full w8 groups,
    padded w8 (r>=5), padded w4 (1<=r<=4).  Empty pillars: no groups."""
    order = np.argsort(pid, kind="stable")
    starts = np.zeros(NPIL, dtype=np.int64)
    np.cumsum(counts[:-1], out=starts[1:])
    r = counts % GK
    n8full = counts // GK
    n8 = n8full + (r >= 5)
    has4 = (r >= 1) & (r <= 4)

    G8 = int(n8.sum())
    pil8 = np.repeat(np.arange(NPIL), n8)
    rank8 = np.arange(G8) - np.repeat(np.cumsum(n8) - n8, n8)
    base8 = starts[pil8]
    cnt8 = counts[pil8]
    m = np.arange(GK)
    offs8 = np.minimum(rank8[:, None] * GK + m[None, :], (cnt8 - 1)[:, None])
    src8 = order[base8[:, None] + offs8]

    pil4 = np.flatnonzero(has4)
    base4 = starts[pil4]
    cnt4 = counts[pil4]
    n8f4 = n8full[pil4]
    m4 = np.arange(4)
    offs4 = np.minimum(n8f4[:, None] * GK + m4[None, :], (cnt4 - 1)[:, None])
    src4 = order[base4[:, None] + offs4]
    return src8, pil8, src4, pil4


def _layout_maps(steps):
    """Per-class slot maps for the given step list.

    Class 8: col8 [S8,8], rb8 [S8,8], oc8 [S8,4], l8 [S8].
    Class 4: col4 [S4,4], rb4 [S4,4], oc4 [S4,2], l4 [S4].
    Slots ordered by step, then within-step index.
    """
    ob_of, wout = _step_offsets(steps)
    m = np.arange(GK)
    m4 = np.arange(4)
    c8 = []
    r8 = []
    o8 = []
    li8 = []
    c4 = []
    r4 = []
    o4 = []
    li4 = []
    for k, st in enumerate(steps):
        bc = 1024 * k
        bo = ob_of[k]
        if st in ("AA", "AB"):
            l = np.repeat([0, 1], 128)
            g = np.tile(np.arange(128), 2)
            c8.append((bc + 8 * g)[:, None] + m[None, :])
            r8.append(np.broadcast_to(l[:, None], (256, GK)).copy())
            o8.append(np.broadcast_to((bo + g)[:, None], (256, 4)).copy())
            li8.append(l)
            if st == "AA":
                c8.append((bc + 8 * g)[:, None] + m[None, :])
                r8.append(np.broadcast_to(2 + l[:, None], (256, GK)).copy())
                o8.append(np.broadcast_to((bo + 128 + g)[:, None],
                                          (256, 4)).copy())
                li8.append(l)
            else:
                c8.append(bc + 128 * m[None, :] + g[:, None])
                r8.append(np.broadcast_to(2 + l[:, None], (256, GK)).copy())
                o8.append((bo + 128 + g)[:, None] + 128 * m4[None, :])
                li8.append(l)
        elif st == "B2":
            l = np.repeat([0, 1], 256)
            j = np.tile(np.arange(256), 2)
            colm = np.where(m[None, :] < 4, 256 * m[None, :],
                            256 * (m[None, :] - 4))
            c8.append(bc + colm + j[:, None])
            r8.append(np.where(m[None, :] < 4, 0, 2) + l[:, None])
            o8.append((bo + j)[:, None] + 256 * m4[None, :])
            li8.append(l)
        else:  # W4
            l = np.repeat([0, 1], 512)
            j = np.tile(np.arange(512), 2)
            colm4 = np.where(m4[None, :] < 2, 512 * m4[None, :],
                             512 * (m4[None, :] - 2))
            c4.append(bc + colm4 + j[:, None])
            r4.append(np.where(m4[None, :] < 2, 0, 2) + l[:, None])
            o4.append((bo + j)[:, None] + 512 * np.arange(2)[None, :])
            li4.append(l)

    def cat2(lst, w):
        return (np.concatenate(lst, axis=0) if lst
                else np.zeros((0, w), np.int64))

    def cat1(lst):
        return np.concatenate(lst) if lst else np.zeros(0, np.int64)

    return {
        "col8": cat2(c8, GK), "rb8": cat2(r8, GK), "oc8": cat2(o8, 4),
        "l8": cat1(li8),
        "col4": cat2(c4, 4), "rb4": cat2(r4, 4), "oc4": cat2(o4, 2),
        "l4": cat1(li4),
    }


def kernel(xyz, xyz_batch_cnt, point_features, conv_w, bn_gamma, bn_beta,
           bn_mean, bn_var):
    from concourse.bass_utils import run_bass_kernel_spmd

    xyz = np.asarray(xyz, dtype=np.float32)
    cnt = np.asarray(xyz_batch_cnt, dtype=np.int64)
    pf = np.asarray(point_features, dtype=np.float32)
    conv_w = np.asarray(conv_w, dtype=np.float32)
    bn_gamma = np.asarray(bn_gamma, dtype=np.float32)
    bn_beta = np.asarray(bn_beta, dtype=np.float32)
    bn_mean = np.asarray(bn_mean, dtype=np.float32)
    bn_var = np.asarray(bn_var, dtype=np.float32)
    N = xyz.shape[0]

    ids = np.repeat(np.arange(B), np.maximum(cnt, 0))
    if ids.shape[0] < N:
        pad_val = ids[-1] if ids.shape[0] else 0
        ids = np.concatenate([ids, np.full(N - ids.shape[0], pad_val, np.int64)])
    ids = ids[:N]

    ix = np.clip(np.floor((xyz[:, 0] - X_MIN) / BEV).astype(np.int32), 0, W - 1)
    iy = np.clip(np.floor((xyz[:, 1] - Y_MIN) / BEV).astype(np.int32), 0, H - 1)
    pid_local = iy.astype(np.int64) * W + ix.astype(np.int64)
    cx = (ix.astype(np.float32) + np.float32(0.5)) * BEV + X_MIN
    cy = (iy.astype(np.float32) + np.float32(0.5)) * BEV + Y_MIN
    feats = np.empty((N, 32), dtype=np.float32)
    feats[:, 0] = xyz[:, 0] - cx
    feats[:, 1] = xyz[:, 1] - cy
    feats[:, 2] = xyz[:, 2]
    feats[:, 3:] = pf

    s = bn_gamma / np.sqrt(bn_var + EPS)
    wt = (conv_w * s[:, None]).T                            # [32, 64]
    wcm = np.zeros((128, 128), dtype=np.float32)
    wcm[0:32, 0:64] = wt
    wcm[32:64, 64:128] = wt
    wcm[64:96, 0:64] = wt
    wcm[96:128, 64:128] = wt
    wcm = wcm.astype(BF16)
    b2 = bn_beta - bn_mean * s

    bounds = np.searchsorted(ids, np.arange(B + 1))
    cores = []
    max8 = max4 = 0
    for c in range(B):
        lo, hi = int(bounds[c]), int(bounds[c + 1])
        pidc = pid_local[lo:hi]
        counts = np.bincount(pidc, minlength=NPIL).astype(np.int64)
        src8, pil8, src4, pil4 = _group_layout(pidc, counts)
        cores.append((lo, hi, src8, pil8, src4, pil4, counts))
        max8 = max(max8, src8.shape[0])
        max4 = max(max4, src4.shape[0])

    n8_steps = math.ceil(max8 / 512)
    n4_steps = math.ceil(max4 / 1024)
    steps = _make_steps(n8_steps, n4_steps)
    cw = len(steps) * 1024

    maps = _layout_maps(steps)
    S8 = maps["col8"].shape[0]
    S4 = maps["col4"].shape[0]
    assert S8 >= max8 and S4 >= max4, (S8, max8, S4, max4)

    if steps not in _prog_cache:
        _prog_cache[steps] = _build_program(steps)
    nc = _prog_cache[steps]

    in_maps = []
    for c in range(B):
        lo, hi, src8, pil8, src4, pil4, counts = cores[c]
        fc = feats[lo:hi]
        if fc.shape[0] == 0:
            fc = np.zeros((1, 32), dtype=np.float32)
        g8 = src8.shape[0]
        g4 = src4.shape[0]
        pts = np.zeros((4, cw), dtype=np.int64)
        pts[maps["rb8"][:g8], maps["col8"][:g8]] = src8
        if g4:
            pts[maps["rb4"][:g4], maps["col4"][:g4]] = src4
        feats_cm = np.empty((128, cw), dtype=np.float32)
        for rb in range(4):
            feats_cm[32 * rb:32 * (rb + 1)] = fc[pts[rb]].T
        in_maps.append({"feats": feats_cm.astype(BF16), "wcm": wcm})

    _debug_state["nc"] = nc
    _debug_state["in_maps"] = in_maps
    res = run_bass_kernel_spmd(nc, in_maps, core_ids=list(range(B)))

    out_full = np.zeros((B * NPIL, C_OUT), dtype=np.float32)
    for c in range(B):
        lo, hi, src8, pil8, src4, pil4, counts = cores[c]
        resr = np.asarray(res.results[c]["outr"]).astype(np.float32)
        resT = resr.T
        g8 = src8.shape[0]
        g4 = src4.shape[0]
        rows = np.empty((g8 + g4, C_OUT), dtype=np.float32)
        oc8 = maps["oc8"][:g8]
        l8 = maps["l8"][:g8]
        for lv in (0, 1):
            msk = l8 == lv
            acc = resT[oc8[msk, 0], 64 * lv:64 * lv + 64]
            for mm in range(1, 4):
                acc = np.maximum(acc, resT[oc8[msk, mm],
                                           64 * lv:64 * lv + 64])
            rows[:g8][msk] = acc
        if g4:
            oc4 = maps["oc4"][:g4]
            l4 = maps["l4"][:g4]
            for lv in (0, 1):
                msk = l4 == lv
                acc = np.maximum(resT[oc4[msk, 0], 64 * lv:64 * lv + 64],
                                 resT[oc4[msk, 1], 64 * lv:64 * lv + 64])
                rows[g8:][msk] = acc
        # per-pillar combine
        allpil = np.concatenate([pil8, pil4])
        order = np.argsort(allpil, kind="stable")
        sp = allpil[order]
        sr = rows[order]
        runs = np.flatnonzero(np.diff(sp, prepend=-1))
        red = np.maximum.reduceat(sr, runs, axis=0)
        upil = sp[runs]
        outc = np.zeros((NPIL, C_OUT), dtype=np.float32)
        outc[upil] = np.maximum(red + b2[None, :], np.float32(0.0))
        outc[counts == 0] = 0.0
        out_full[c * NPIL:(c + 1) * NPIL] = outc
    return out_full


# revision 28
# speedup vs baseline: 1.0026x; 1.0026x over previous
"""PillarMaxPoolingV2a on 8 TRN2 NeuronCores (Bass/Tile) - v12.

Device architecture (per core = one batch, ~213k padded points):
  - feats bf16 channel-major [128, CW]: column c packs 4 points:
    rows 0:32   = channels of point (s=0, l=0), rows 32:64 = (s=0, l=1)
    rows 64:96  = (s=1, l=0), rows 96:128 = (s=1, l=1)
  - block-diagonal W [128, 128] bf16; stream s matmul: lhsT=wcm[64s:64s+64],
    rhs=f[64s:64s+64, cols] -> PSUM [128, cols]: rows 0:64 = 64 out-channels
    of the l=0 point of each column, rows 64:128 = l=1 point.  The two
    streams run on disjoint PE row-quadrants.
  - The kernel is a flat list of STEPS (1024 feats cols = 4096 points each;
    two [128,1024] fp32 PSUM duos from a 4-buf pool).  Step types:
    "AA": both duos window-8 tensor_reduce (DVE, from PSUM), members
          consecutive (col 8g+m).  2 x 128 out cols, fully reduced.
    "AB": s0 like AA; s1: ScalarE copy -> SBUF bf16, members self-
          interleaved (col 128m+g), one contiguous-halves TT max (2x mode)
          -> 4 partial cols per group (host folds).  128 + 512 out cols.
    "B2": 512 w8-groups pair-interleaved across both duos (member m of
          group (l,j): m<4 at s0-col 256m+j, m>=4 at s1-col 256(m-4)+j);
          two ScalarE copies into one [128,2048] collect, one TT max
          -> 4 partials per group.  1024 out cols.
    "W4": 1024 window-4 groups (pillar tails, r in 1..4, padded to 4):
          member m<2 at s0-col 512m+j, m>=2 at s1-col 512(m-2)+j; two
          copies + one TT -> 2 partials per group.  1024 out cols.
  - Tree TTs are deferred one step so the DVE queue never head-of-line
    blocks on a ScalarE copy.  feats DMA per 2 steps; output staged per
    4 steps, DMA'd out via GPSIMD SWDGE (separate queue from input).
Host: pillar sort; per pillar floor(c/8) full w8-groups, one padded
w8-group if r>=5, one padded w4-group if 1<=r<=4 (empty pillars get no
group).  Gather/scatter layouts above; final fold + per-pillar combine +
bias + ReLU + masking on host.
"""
import math
import numpy as np
import sys

sys.path.insert(0, "/opt/trn_rl_repo")

import ml_dtypes

BF16 = ml_dtypes.bfloat16

# ---- problem constants (hardcoded per contract) ----
B = 8
NPOINTS = 1_600_000
C_IN = 29
C_MLP0 = 32
C_OUT = 64
BEV = np.float32(0.8)
X_MIN = np.float32(0.0)
Y_MIN = np.float32(-40.0)
Z_MIN = np.float32(-3.0)
W = 88
H = 100
EPS = np.float32(1e-5)
NPIL = H * W            # 8800 pillars per batch
GK = 8

# w8-step mix pattern (cycled); W4 steps are interleaved separately
P8 = ("AA", "B2", "AB", "B2", "AA", "B2", "B2", "AA")
STEP_W = {"AA": 256, "AB": 640, "B2": 1024, "W4": 1024}

_prog_cache = {}
_debug_state = {}


def _make_steps(n8, n4):
    """Interleave n8 w8-steps (cycling P8) with n4 W4 steps, spread out."""
    steps = []
    i8 = 0
    if n4:
        gap = max(1, round(n8 / n4))
        for i4 in range(n4):
            take = min(gap, n8 - i8)
            for _ in range(take):
                steps.append(P8[i8 % len(P8)])
                i8 += 1
            steps.append("W4")
        while i8 < n8:
            steps.append(P8[i8 % len(P8)])
            i8 += 1
    else:
        steps = [P8[i % len(P8)] for i in range(n8)]
    # drain optimization: end with AA steps (V-direct, no collect->tree
    # chain), so the final stage flushes quickly
    tail = steps[-8:]
    steps[-8:] = [s for s in tail if s != "AA"] + [s for s in tail if s == "AA"]
    return tuple(steps)


def _step_offsets(steps):
    """Per-step output col base and total output width."""
    ob = []
    off = 0
    for st in steps:
        ob.append(off)
        off += STEP_W[st]
    return ob, off


def _build_program(steps):
    """Device program for the given step list."""
    import concourse.bass as bass
    import concourse.bacc as bacc
    import concourse.mybir as mybir
    import concourse.tile as tile
    import contextlib

    MAX = mybir.AluOpType.max
    X = mybir.AxisListType.X
    nst = len(steps)
    ob_of, wout = _step_offsets(steps)
    cw = nst * 1024

    nc = bacc.Bacc("TRN2", target_bir_lowering=False, debug=False, num_devices=8)
    feats_in = nc.declare_dram_parameter("feats", [128, cw], mybir.dt.bfloat16,
                                         isOutput=False)
    w_in = nc.declare_dram_parameter("wcm", [128, 128], mybir.dt.bfloat16,
                                     isOutput=False)
    out = nc.declare_dram_parameter("outr", [128, wout],
                                    mybir.dt.bfloat16, isOutput=True)

    with tile.TileContext(nc) as tc:
        with contextlib.ExitStack() as stack:
            constp = stack.enter_context(tc.tile_pool(name="const", bufs=1))
            featsp = stack.enter_context(tc.tile_pool(name="feats", bufs=6))
            psump = stack.enter_context(
                tc.tile_pool(name="psum", bufs=4, space="PSUM"))
            bcollp = stack.enter_context(tc.tile_pool(name="bcoll", bufs=4))
            b2collp = stack.enter_context(tc.tile_pool(name="b2coll", bufs=3))
            stgap = stack.enter_context(tc.tile_pool(name="stga", bufs=3))

            wcm = constp.tile([128, 128], mybir.dt.bfloat16)
            nc.sync.dma_start(out=wcm[:], in_=w_in[:])

            # PE warm-up: ~5.6us of back-to-back matmuls during the boot
            # window flips the HAM clock gate to K=8/8 before real work;
            # steady-state PE duty then keeps it warm.
            warm = psump.tile([128, 1024], mybir.dt.float32, tag="duo")
            for _ in range(26):
                nc.tensor.matmul(
                    out=warm[:, 0:128],
                    lhsT=wcm[0:64, :],
                    rhs=wcm[0:64, :],
                    start=True, stop=True,
                )

            def emit_tree(job):
                # one contiguous-halves TT max; host folds the partials
                bc, n, dst = job
                h = n // 2
                nc.vector.tensor_tensor(
                    out=dst, in0=bc[:, 0:h], in1=bc[:, h:n], op=MAX)

            fh = None
            stga = None
            stga_done = 0
            stga_base = 0
            pending = []
            for k, st in enumerate(steps):
                if k % 2 == 0:
                    nh = min(2, nst - k)
                    fh = featsp.tile([128, 2048], mybir.dt.bfloat16,
                                     tag="ftile")
                    nc.sync.dma_start(
                        out=fh[:, 0:1024 * nh],
                        in_=feats_in[:, 1024 * k:1024 * (k + nh)])
                if stga is None:
                    lastk = min(k + 4, nst)
                    pw = (ob_of[lastk] if lastk < nst else wout) - ob_of[k]
                    stga_base = ob_of[k]
                    stga = stgap.tile([128, pw], mybir.dt.bfloat16, tag="stga")
                    stga_done = lastk

                c0 = 1024 * (k % 2)
                ob = ob_of[k] - stga_base
                duos = []
                for s in range(2):
                    p = psump.tile([128, 1024], mybir.dt.float32, tag="duo")
                    duos.append(p)
                for jj in range(2):
                    for s in range(2):
                        nc.tensor.matmul(
                            out=duos[s][:, 512 * jj:512 * (jj + 1)],
                            lhsT=wcm[64 * s:64 * (s + 1), :],
                            rhs=fh[64 * s:64 * (s + 1),
                                   c0 + 512 * jj:c0 + 512 * (jj + 1)],
                            start=True, stop=True,
                        )
                if st == "AA" or st == "AB":
                    nc.vector.tensor_reduce(
                        out=stga[:, ob:ob + 128],
                        in_=duos[0][:].rearrange("p (g k) -> p g k", k=GK),
                        axis=X, op=MAX,
                    )
                    if st == "AA":
                        nc.vector.tensor_reduce(
                            out=stga[:, ob + 128:ob + 256],
                            in_=duos[1][:].rearrange("p (g k) -> p g k", k=GK),
                            axis=X, op=MAX,
                        )
                    else:
                        bc = bcollp.tile([128, 1024], mybir.dt.bfloat16,
                                         tag="bcl")
                        nc.scalar.copy(out=bc[:], in_=duos[1][:])
                        pending.append((bc, 1024, stga[:, ob + 128:ob + 640]))
                else:  # B2 / W4
                    bc = b2collp.tile([128, 2048], mybir.dt.bfloat16,
                                      tag="b2cl")
                    nc.scalar.copy(out=bc[:, 0:1024], in_=duos[0][:])
                    nc.scalar.copy(out=bc[:, 1024:2048], in_=duos[1][:])
                    pending.append((bc, 2048, stga[:, ob:ob + 1024]))
                while len(pending) > 1:
                    emit_tree(pending.pop(0))

                if k + 1 == stga_done:
                    while pending:
                        emit_tree(pending.pop(0))
                    nc.gpsimd.dma_start(
                        out=out[:, stga_base:stga_base + stga.shape[1]],
                        in_=stga[:],
                    )
                    stga = None
    nc.compile()
    return nc


def _group_layout(pid, counts):
    """Two-class grouping.  Returns (src8 [G8,8], pil8 [G8],
    src4 [G4,4], pil4 [G4]) with per-pillar order: full w8 groups,
    padded w8 (r>=5), padded w4 (1<=r<=4).  Empty pillars: no groups."""
    order = np.argsort(pid, kind="stable")
    starts = np.zeros(NPIL, dtype=np.int64)
    np.cumsum(counts[:-1], out=starts[1:])
    r = counts % GK
    n8full = counts // GK
    n8 = n8full + (r >= 5)
    has4 = (r >= 1) & (r <= 4)

    G8 = int(n8.sum())
    pil8 = np.repeat(np.arange(NPIL), n8)
    rank8 = np.arange(G8) - np.repeat(np.cumsum(n8) - n8, n8)
    base8 = starts[pil8]
    cnt8 = counts[pil8]
    m = np.arange(GK)
    offs8 = np.minimum(rank8[:, None] * GK + m[None, :], (cnt8 - 1)[:, None])
    src8 = order[base8[:, None] + offs8]

    pil4 = np.flatnonzero(has4)
    base4 = starts[pil4]
    cnt4 = counts[pil4]
    n8f4 = n8full[pil4]
    m4 = np.arange(4)
    offs4 = np.minimum(n8f4[:, None] * GK + m4[None, :], (cnt4 - 1)[:, None])
    src4 = order[base4[:, None] + offs4]
    return src8, pil8, src4, pil4


def _layout_maps(steps):
    """Per-class slot maps for the given step list.

    Class 8: col8 [S8,8], rb8 [S8,8], oc8 [S8,4], l8 [S8].
    Class 4: col4 [S4,4], rb4 [S4,4], oc4 [S4,2], l4 [S4].
    Slots ordered by step, then within-step index.
    """
    ob_of, wout = _step_offsets(steps)
    m = np.arange(GK)
    m4 = np.arange(4)
    c8 = []
    r8 = []
    o8 = []
    li8 = []
    c4 = []
    r4 = []
    o4 = []
    li4 = []
    for k, st in enumerate(steps):
        bc = 1024 * k
        bo = ob_of[k]
        if st in ("AA", "AB"):
            l = np.repeat([0, 1], 128)
            g = np.tile(np.arange(128), 2)
            c8.append((bc + 8 * g)[:, None] + m[None, :])
            r8.append(np.broadcast_to(l[:, None], (256, GK)).copy())
            o8.append(np.broadcast_to((bo + g)[:, None], (256, 4)).copy())
            li8.append(l)
            if st == "AA":
                c8.append((bc + 8 * g)[:, None] + m[None, :])
                r8.append(np.broadcast_to(2 + l[:, None], (256, GK)).copy())
                o8.append(np.broadcast_to((bo + 128 + g)[:, None],
                                          (256, 4)).copy())
                li8.append(l)
            else:
                c8.append(bc + 128 * m[None, :] + g[:, None])
                r8.append(np.broadcast_to(2 + l[:, None], (256, GK)).copy())
                o8.append((bo + 128 + g)[:, None] + 128 * m4[None, :])
                li8.append(l)
        elif st == "B2":
            l = np.repeat([0, 1], 256)
            j = np.tile(np.arange(256), 2)
            colm = np.where(m[None, :] < 4, 256 * m[None, :],
                            256 * (m[None, :] - 4))
            c8.append(bc + colm + j[:, None])
            r8.append(np.where(m[None, :] < 4, 0, 2) + l[:, None])
            o8.append((bo + j)[:, None] + 256 * m4[None, :])
            li8.append(l)
        else:  # W4
            l = np.repeat([0, 1], 512)
            j = np.tile(np.arange(512), 2)
            colm4 = np.where(m4[None, :] < 2, 512 * m4[None, :],
                             512 * (m4[None, :] - 2))
            c4.append(bc + colm4 + j[:, None])
            r4.append(np.where(m4[None, :] < 2, 0, 2) + l[:, None])
            o4.append((bo + j)[:, None] + 512 * np.arange(2)[None, :])
            li4.append(l)

    def cat2(lst, w):
        return (np.concatenate(lst, axis=0) if lst
                else np.zeros((0, w), np.int64))

    def cat1(lst):
        return np.concatenate(lst) if lst else np.zeros(0, np.int64)

    return {
        "col8": cat2(c8, GK), "rb8": cat2(r8, GK), "oc8": cat2(o8, 4),
        "l8": cat1(li8),
        "col4": cat2(c4, 4), "rb4": cat2(r4, 4), "oc4": cat2(o4, 2),
        "l4": cat1(li4),
    }


def kernel(xyz, xyz_batch_cnt, point_features, conv_w, bn_gamma, bn_beta,
           bn_mean, bn_var):
    from concourse.bass_utils import run_bass_kernel_spmd

    xyz = np.asarray(xyz, dtype=np.float32)
    cnt = np.asarray(xyz_batch_cnt, dtype=np.int64)
    pf = np.asarray(point_features, dtype=np.float32)
    conv_w = np.asarray(conv_w, dtype=np.float32)
    bn_gamma = np.asarray(bn_gamma, dtype=np.float32)
    bn_beta = np.asarray(bn_beta, dtype=np.float32)
    bn_mean = np.asarray(bn_mean, dtype=np.float32)
    bn_var = np.asarray(bn_var, dtype=np.float32)
    N = xyz.shape[0]

    ids = np.repeat(np.arange(B), np.maximum(cnt, 0))
    if ids.shape[0] < N:
        pad_val = ids[-1] if ids.shape[0] else 0
        ids = np.concatenate([ids, np.full(N - ids.shape[0], pad_val, np.int64)])
    ids = ids[:N]

    ix = np.clip(np.floor((xyz[:, 0] - X_MIN) / BEV).astype(np.int32), 0, W - 1)
    iy = np.clip(np.floor((xyz[:, 1] - Y_MIN) / BEV).astype(np.int32), 0, H - 1)
    pid_local = iy.astype(np.int64) * W + ix.astype(np.int64)
    cx = (ix.astype(np.float32) + np.float32(0.5)) * BEV + X_MIN
    cy = (iy.astype(np.float32) + np.float32(0.5)) * BEV + Y_MIN
    feats = np.empty((N, 32), dtype=np.float32)
    feats[:, 0] = xyz[:, 0] - cx
    feats[:, 1] = xyz[:, 1] - cy
    feats[:, 2] = xyz[:, 2]
    feats[:, 3:] = pf

    s = bn_gamma / np.sqrt(bn_var + EPS)
    wt = (conv_w * s[:, None]).T                            # [32, 64]
    wcm = np.zeros((128, 128), dtype=np.float32)
    wcm[0:32, 0:64] = wt
    wcm[32:64, 64:128] = wt
    wcm[64:96, 0:64] = wt
    wcm[96:128, 64:128] = wt
    wcm = wcm.astype(BF16)
    b2 = bn_beta - bn_mean * s

    bounds = np.searchsorted(ids, np.arange(B + 1))
    cores = []
    max8 = max4 = 0
    for c in range(B):
        lo, hi = int(bounds[c]), int(bounds[c + 1])
        pidc = pid_local[lo:hi]
        counts = np.bincount(pidc, minlength=NPIL).astype(np.int64)
        src8, pil8, src4, pil4 = _group_layout(pidc, counts)
        cores.append((lo, hi, src8, pil8, src4, pil4, counts))
        max8 = max(max8, src8.shape[0])
        max4 = max(max4, src4.shape[0])

    n8_steps = math.ceil(max8 / 512)
    n4_steps = math.ceil(max4 / 1024)
    steps = _make_steps(n8_steps, n4_steps)
    cw = len(steps) * 1024

    maps = _layout_maps(steps)
    S8 = maps["col8"].shape[0]
    S4 = maps["col4"].shape[0]
    assert S8 >= max8 and S4 >= max4, (S8, max8, S4, max4)

    if steps not in _prog_cache:
        _prog_cache[steps] = _build_program(steps)
    nc = _prog_cache[steps]

    in_maps = []
    for c in range(B):
        lo, hi, src8, pil8, src4, pil4, counts = cores[c]
        fc = feats[lo:hi]
        if fc.shape[0] == 0:
            fc = np.zeros((1, 32), dtype=np.float32)
        g8 = src8.shape[0]
        g4 = src4.shape[0]
        pts = np.zeros((4, cw), dtype=np.int64)
        pts[maps["rb8"][:g8], maps["col8"][:g8]] = src8
        if g4:
            pts[maps["rb4"][:g4], maps["col4"][:g4]] = src4
        feats_cm = np.empty((128, cw), dtype=np.float32)
        for rb in range(4):
            feats_cm[32 * rb:32 * (rb + 1)] = fc[pts[rb]].T
        in_maps.append({"feats": feats_cm.astype(BF16), "wcm": wcm})

    _debug_state["nc"] = nc
    _debug_state["in_maps"] = in_maps
    res = run_bass_kernel_spmd(nc, in_maps, core_ids=list(range(B)))

    out_full = np.zeros((B * NPIL, C_OUT), dtype=np.float32)
    for c in range(B):
        lo, hi, src8, pil8, src4, pil4, counts = cores[c]
        resr = np.asarray(res.results[c]["outr"]).astype(np.float32)
        resT = resr.T
        g8 = src8.shape[0]
        g4 = src4.shape[0]
        rows = np.empty((g8 + g4, C_OUT), dtype=np.float32)
        oc8 = maps["oc8"][:g8]
        l8 = maps["l8"][:g8]
        for lv in (0, 1):
            msk = l8 == lv
            acc = resT[oc8[msk, 0], 64 * lv:64 * lv + 64]
            for mm in range(1, 4):
                acc = np.maximum(acc, resT[oc8[msk, mm],
                                           64 * lv:64 * lv + 64])
            rows[:g8][msk] = acc
        if g4:
            oc4 = maps["oc4"][:g4]
            l4 = maps["l4"][:g4]
            for lv in (0, 1):
                msk = l4 == lv
                acc = np.maximum(resT[oc4[msk, 0], 64 * lv:64 * lv + 64],
                                 resT[oc4[msk, 1], 64 * lv:64 * lv + 64])
                rows[g8:][msk] = acc
        # per-pillar combine
        allpil = np.concatenate([pil8, pil4])
        order = np.argsort(allpil, kind="stable")
        sp = allpil[order]
        sr = rows[order]
        runs = np.flatnonzero(np.diff(sp, prepend=-1))
        red = np.maximum.reduceat(sr, runs, axis=0)
        upil = sp[runs]
        outc = np.zeros((NPIL, C_OUT), dtype=np.float32)
        outc[upil] = np.maximum(red + b2[None, :], np.float32(0.0))
        outc[counts == 0] = 0.0
        out_full[c * NPIL:(c + 1) * NPIL] = outc
    return out_full


# revision 30
# speedup vs baseline: 1.0130x; 1.0104x over previous
"""PillarMaxPoolingV2a on 8 TRN2 NeuronCores (Bass/Tile) - v12.

Device architecture (per core = one batch, ~213k padded points):
  - feats bf16 channel-major [128, CW]: column c packs 4 points:
    rows 0:32   = channels of point (s=0, l=0), rows 32:64 = (s=0, l=1)
    rows 64:96  = (s=1, l=0), rows 96:128 = (s=1, l=1)
  - block-diagonal W [128, 128] bf16; stream s matmul: lhsT=wcm[64s:64s+64],
    rhs=f[64s:64s+64, cols] -> PSUM [128, cols]: rows 0:64 = 64 out-channels
    of the l=0 point of each column, rows 64:128 = l=1 point.  The two
    streams run on disjoint PE row-quadrants.
  - The kernel is a flat list of STEPS (1024 feats cols = 4096 points each;
    two [128,1024] fp32 PSUM duos from a 4-buf pool).  Step types:
    "AA": both duos window-8 tensor_reduce (DVE, from PSUM), members
          consecutive (col 8g+m).  2 x 128 out cols, fully reduced.
    "AB": s0 like AA; s1: ScalarE copy -> SBUF bf16, members self-
          interleaved (col 128m+g), one contiguous-halves TT max (2x mode)
          -> 4 partial cols per group (host folds).  128 + 512 out cols.
    "B2": 512 w8-groups pair-interleaved across both duos (member m of
          group (l,j): m<4 at s0-col 256m+j, m>=4 at s1-col 256(m-4)+j);
          two ScalarE copies into one [128,2048] collect, one TT max
          -> 4 partials per group.  1024 out cols.
    "W4": 1024 window-4 groups (pillar tails, r in 1..4, padded to 4):
          member m<2 at s0-col 512m+j, m>=2 at s1-col 512(m-2)+j; two
          copies + one TT -> 2 partials per group.  1024 out cols.
  - Tree TTs are deferred one step so the DVE queue never head-of-line
    blocks on a ScalarE copy.  feats DMA per 2 steps; output staged per
    4 steps, DMA'd out via GPSIMD SWDGE (separate queue from input).
Host: pillar sort; per pillar floor(c/8) full w8-groups, one padded
w8-group if r>=5, one padded w4-group if 1<=r<=4 (empty pillars get no
group).  Gather/scatter layouts above; final fold + per-pillar combine +
bias + ReLU + masking on host.
"""
import math
import numpy as np
import sys

sys.path.insert(0, "/opt/trn_rl_repo")

import ml_dtypes

BF16 = ml_dtypes.bfloat16

# ---- problem constants (hardcoded per contract) ----
B = 8
NPOINTS = 1_600_000
C_IN = 29
C_MLP0 = 32
C_OUT = 64
BEV = np.float32(0.8)
X_MIN = np.float32(0.0)
Y_MIN = np.float32(-40.0)
Z_MIN = np.float32(-3.0)
W = 88
H = 100
EPS = np.float32(1e-5)
NPIL = H * W            # 8800 pillars per batch
GK = 8

# w8-step mix pattern (cycled); W4 steps are interleaved separately
P8 = ("AA", "B2", "AB", "B2", "AA", "B2", "B2", "AA")
STEP_W = {"AA": 256, "AB": 640, "B2": 1024, "W4": 1024}

_prog_cache = {}
_debug_state = {}


def _make_steps(n8, n4):
    """Interleave n8 w8-steps (cycling P8) with n4 W4 steps, spread out."""
    steps = []
    i8 = 0
    if n4:
        gap = max(1, round(n8 / n4))
        for i4 in range(n4):
            take = min(gap, n8 - i8)
            for _ in range(take):
                steps.append(P8[i8 % len(P8)])
                i8 += 1
            steps.append("W4")
        while i8 < n8:
            steps.append(P8[i8 % len(P8)])
            i8 += 1
    else:
        steps = [P8[i % len(P8)] for i in range(n8)]
    # drain optimization: end with AA steps (V-direct, no collect->tree
    # chain), so the final stage flushes quickly
    tail = steps[-8:]
    steps[-8:] = [s for s in tail if s != "AA"] + [s for s in tail if s == "AA"]
    return tuple(steps)


def _step_offsets(steps):
    """Per-step output col base and total output width."""
    ob = []
    off = 0
    for st in steps:
        ob.append(off)
        off += STEP_W[st]
    return ob, off


def _build_program(steps):
    """Device program for the given step list."""
    import concourse.bass as bass
    import concourse.bacc as bacc
    import concourse.mybir as mybir
    import concourse.tile as tile
    import contextlib

    MAX = mybir.AluOpType.max
    X = mybir.AxisListType.X
    nst = len(steps)
    ob_of, wout = _step_offsets(steps)
    cw = nst * 1024

    nc = bacc.Bacc("TRN2", target_bir_lowering=False, debug=False, num_devices=8)
    feats_in = nc.declare_dram_parameter("feats", [128, cw], mybir.dt.bfloat16,
                                         isOutput=False)
    w_in = nc.declare_dram_parameter("wcm", [128, 128], mybir.dt.bfloat16,
                                     isOutput=False)
    out = nc.declare_dram_parameter("outr", [128, wout],
                                    mybir.dt.bfloat16, isOutput=True)

    with tile.TileContext(nc) as tc:
        with contextlib.ExitStack() as stack:
            constp = stack.enter_context(tc.tile_pool(name="const", bufs=1))
            featsp = stack.enter_context(tc.tile_pool(name="feats", bufs=6))
            psump = stack.enter_context(
                tc.tile_pool(name="psum", bufs=4, space="PSUM"))
            bcollp = stack.enter_context(tc.tile_pool(name="bcoll", bufs=4))
            b2collp = stack.enter_context(tc.tile_pool(name="b2coll", bufs=3))
            stgap = stack.enter_context(tc.tile_pool(name="stga", bufs=3))

            wcm = constp.tile([128, 128], mybir.dt.bfloat16)
            nc.sync.dma_start(out=wcm[:], in_=w_in[:])

            # PE warm-up: ~5.6us of back-to-back matmuls during the boot
            # window flips the HAM clock gate to K=8/8 before real work;
            # steady-state PE duty then keeps it warm.
            warm = psump.tile([128, 1024], mybir.dt.float32, tag="duo")
            for _ in range(26):
                nc.tensor.matmul(
                    out=warm[:, 0:128],
                    lhsT=wcm[0:64, :],
                    rhs=wcm[0:64, :],
                    start=True, stop=True,
                )

            def emit_tree(job):
                # one contiguous-halves TT max; host folds the partials
                bc, n, dst = job
                h = n // 2
                nc.vector.tensor_tensor(
                    out=dst, in0=bc[:, 0:h], in1=bc[:, h:n], op=MAX)

            fh = None
            stga = None
            stga_done = 0
            stga_base = 0
            pending = []
            for k, st in enumerate(steps):
                if k % 2 == 0:
                    nh = min(2, nst - k)
                    fh = featsp.tile([128, 2048], mybir.dt.bfloat16,
                                     tag="ftile")
                    nc.sync.dma_start(
                        out=fh[:, 0:1024 * nh],
                        in_=feats_in[:, 1024 * k:1024 * (k + nh)])
                if stga is None:
                    lastk = min(k + 4, nst)
                    pw = (ob_of[lastk] if lastk < nst else wout) - ob_of[k]
                    stga_base = ob_of[k]
                    stga = stgap.tile([128, pw], mybir.dt.bfloat16, tag="stga")
                    stga_done = lastk

                c0 = 1024 * (k % 2)
                ob = ob_of[k] - stga_base
                duos = []
                for s in range(2):
                    p = psump.tile([128, 1024], mybir.dt.float32, tag="duo")
                    duos.append(p)
                for jj in range(2):
                    for s in range(2):
                        nc.tensor.matmul(
                            out=duos[s][:, 512 * jj:512 * (jj + 1)],
                            lhsT=wcm[64 * s:64 * (s + 1), :],
                            rhs=fh[64 * s:64 * (s + 1),
                                   c0 + 512 * jj:c0 + 512 * (jj + 1)],
                            start=True, stop=True,
                        )
                if st == "AA" or st == "AB":
                    nc.vector.tensor_reduce(
                        out=stga[:, ob:ob + 128],
                        in_=duos[0][:].rearrange("p (g k) -> p g k", k=GK),
                        axis=X, op=MAX,
                    )
                    if st == "AA":
                        nc.vector.tensor_reduce(
                            out=stga[:, ob + 128:ob + 256],
                            in_=duos[1][:].rearrange("p (g k) -> p g k", k=GK),
                            axis=X, op=MAX,
                        )
                    else:
                        bc = bcollp.tile([128, 1024], mybir.dt.bfloat16,
                                         tag="bcl")
                        nc.scalar.copy(out=bc[:], in_=duos[1][:])
                        pending.append((bc, 1024, stga[:, ob + 128:ob + 640]))
                else:  # B2 / W4
                    bc = b2collp.tile([128, 2048], mybir.dt.bfloat16,
                                      tag="b2cl")
                    nc.scalar.copy(out=bc[:, 0:1024], in_=duos[0][:])
                    nc.scalar.copy(out=bc[:, 1024:2048], in_=duos[1][:])
                    pending.append((bc, 2048, stga[:, ob:ob + 1024]))
                while len(pending) > 1:
                    emit_tree(pending.pop(0))

                if k + 1 == stga_done:
                    while pending:
                        emit_tree(pending.pop(0))
                    # last two stages flush via HWDGE (Sync): the
                    # end-of-program SWDGE drain then only waits on DMAs
                    # that finished long before
                    eng = nc.sync if stga_done > nst - 9 else nc.gpsimd
                    eng.dma_start(
                        out=out[:, stga_base:stga_base + stga.shape[1]],
                        in_=stga[:],
                    )
                    stga = None
    nc.compile()
    return nc


def _group_layout(pid, counts):
    """Two-class grouping.  Returns (src8 [G8,8], pil8 [G8],
    src4 [G4,4], pil4 [G4]) with per-pillar order: full w8 groups,
    padded w8 (r>=5), padded w4 (1<=r<=4).  Empty pillars: no groups."""
    order = np.argsort(pid, kind="stable")
    starts = np.zeros(NPIL, dtype=np.int64)
    np.cumsum(counts[:-1], out=starts[1:])
    r = counts % GK
    n8full = counts // GK
    n8 = n8full + (r >= 5)
    has4 = (r >= 1) & (r <= 4)

    G8 = int(n8.sum())
    pil8 = np.repeat(np.arange(NPIL), n8)
    rank8 = np.arange(G8) - np.repeat(np.cumsum(n8) - n8, n8)
    base8 = starts[pil8]
    cnt8 = counts[pil8]
    m = np.arange(GK)
    offs8 = np.minimum(rank8[:, None] * GK + m[None, :], (cnt8 - 1)[:, None])
    src8 = order[base8[:, None] + offs8]

    pil4 = np.flatnonzero(has4)
    base4 = starts[pil4]
    cnt4 = counts[pil4]
    n8f4 = n8full[pil4]
    m4 = np.arange(4)
    offs4 = np.minimum(n8f4[:, None] * GK + m4[None, :], (cnt4 - 1)[:, None])
    src4 = order[base4[:, None] + offs4]
    return src8, pil8, src4, pil4


def _layout_maps(steps):
    """Per-class slot maps for the given step list.

    Class 8: col8 [S8,8], rb8 [S8,8], oc8 [S8,4], l8 [S8].
    Class 4: col4 [S4,4], rb4 [S4,4], oc4 [S4,2], l4 [S4].
    Slots ordered by step, then within-step index.
    """
    ob_of, wout = _step_offsets(steps)
    m = np.arange(GK)
    m4 = np.arange(4)
    c8 = []
    r8 = []
    o8 = []
    li8 = []
    c4 = []
    r4 = []
    o4 = []
    li4 = []
    for k, st in enumerate(steps):
        bc = 1024 * k
        bo = ob_of[k]
        if st in ("AA", "AB"):
            l = np.repeat([0, 1], 128)
            g = np.tile(np.arange(128), 2)
            c8.append((bc + 8 * g)[:, None] + m[None, :])
            r8.append(np.broadcast_to(l[:, None], (256, GK)).copy())
            o8.append(np.broadcast_to((bo + g)[:, None], (256, 4)).copy())
            li8.append(l)
            if st == "AA":
                c8.append((bc + 8 * g)[:, None] + m[None, :])
                r8.append(np.broadcast_to(2 + l[:, None], (256, GK)).copy())
                o8.append(np.broadcast_to((bo + 128 + g)[:, None],
                                          (256, 4)).copy())
                li8.append(l)
            else:
                c8.append(bc + 128 * m[None, :] + g[:, None])
                r8.append(np.broadcast_to(2 + l[:, None], (256, GK)).copy())
                o8.append((bo + 128 + g)[:, None] + 128 * m4[None, :])
                li8.append(l)
        elif st == "B2":
            l = np.repeat([0, 1], 256)
            j = np.tile(np.arange(256), 2)
            colm = np.where(m[None, :] < 4, 256 * m[None, :],
                            256 * (m[None, :] - 4))
            c8.append(bc + colm + j[:, None])
            r8.append(np.where(m[None, :] < 4, 0, 2) + l[:, None])
            o8.append((bo + j)[:, None] + 256 * m4[None, :])
            li8.append(l)
        else:  # W4
            l = np.repeat([0, 1], 512)
            j = np.tile(np.arange(512), 2)
            colm4 = np.where(m4[None, :] < 2, 512 * m4[None, :],
                             512 * (m4[None, :] - 2))
            c4.append(bc + colm4 + j[:, None])
            r4.append(np.where(m4[None, :] < 2, 0, 2) + l[:, None])
            o4.append((bo + j)[:, None] + 512 * np.arange(2)[None, :])
            li4.append(l)

    def cat2(lst, w):
        return (np.concatenate(lst, axis=0) if lst
                else np.zeros((0, w), np.int64))

    def cat1(lst):
        return np.concatenate(lst) if lst else np.zeros(0, np.int64)

    return {
        "col8": cat2(c8, GK), "rb8": cat2(r8, GK), "oc8": cat2(o8, 4),
        "l8": cat1(li8),
        "col4": cat2(c4, 4), "rb4": cat2(r4, 4), "oc4": cat2(o4, 2),
        "l4": cat1(li4),
    }


def kernel(xyz, xyz_batch_cnt, point_features, conv_w, bn_gamma, bn_beta,
           bn_mean, bn_var):
    from concourse.bass_utils import run_bass_kernel_spmd

    xyz = np.asarray(xyz, dtype=np.float32)
    cnt = np.asarray(xyz_batch_cnt, dtype=np.int64)
    pf = np.asarray(point_features, dtype=np.float32)
    conv_w = np.asarray(conv_w, dtype=np.float32)
    bn_gamma = np.asarray(bn_gamma, dtype=np.float32)
    bn_beta = np.asarray(bn_beta, dtype=np.float32)
    bn_mean = np.asarray(bn_mean, dtype=np.float32)
    bn_var = np.asarray(bn_var, dtype=np.float32)
    N = xyz.shape[0]

    ids = np.repeat(np.arange(B), np.maximum(cnt, 0))
    if ids.shape[0] < N:
        pad_val = ids[-1] if ids.shape[0] else 0
        ids = np.concatenate([ids, np.full(N - ids.shape[0], pad_val, np.int64)])
    ids = ids[:N]

    ix = np.clip(np.floor((xyz[:, 0] - X_MIN) / BEV).astype(np.int32), 0, W - 1)
    iy = np.clip(np.floor((xyz[:, 1] - Y_MIN) / BEV).astype(np.int32), 0, H - 1)
    pid_local = iy.astype(np.int64) * W + ix.astype(np.int64)
    cx = (ix.astype(np.float32) + np.float32(0.5)) * BEV + X_MIN
    cy = (iy.astype(np.float32) + np.float32(0.5)) * BEV + Y_MIN
    feats = np.empty((N, 32), dtype=np.float32)
    feats[:, 0] = xyz[:, 0] - cx
    feats[:, 1] = xyz[:, 1] - cy
    feats[:, 2] = xyz[:, 2]
    feats[:, 3:] = pf

    s = bn_gamma / np.sqrt(bn_var + EPS)
    wt = (conv_w * s[:, None]).T                            # [32, 64]
    wcm = np.zeros((128, 128), dtype=np.float32)
    wcm[0:32, 0:64] = wt
    wcm[32:64, 64:128] = wt
    wcm[64:96, 0:64] = wt
    wcm[96:128, 64:128] = wt
    wcm = wcm.astype(BF16)
    b2 = bn_beta - bn_mean * s

    bounds = np.searchsorted(ids, np.arange(B + 1))
    cores = []
    max8 = max4 = 0
    for c in range(B):
        lo, hi = int(bounds[c]), int(bounds[c + 1])
        pidc = pid_local[lo:hi]
        counts = np.bincount(pidc, minlength=NPIL).astype(np.int64)
        src8, pil8, src4, pil4 = _group_layout(pidc, counts)
        cores.append((lo, hi, src8, pil8, src4, pil4, counts))
        max8 = max(max8, src8.shape[0])
        max4 = max(max4, src4.shape[0])

    n8_steps = math.ceil(max8 / 512)
    n4_steps = math.ceil(max4 / 1024)
    steps = _make_steps(n8_steps, n4_steps)
    cw = len(steps) * 1024

    maps = _layout_maps(steps)
    S8 = maps["col8"].shape[0]
    S4 = maps["col4"].shape[0]
    assert S8 >= max8 and S4 >= max4, (S8, max8, S4, max4)

    if steps not in _prog_cache:
        _prog_cache[steps] = _build_program(steps)
    nc = _prog_cache[steps]

    in_maps = []
    for c in range(B):
        lo, hi, src8, pil8, src4, pil4, counts = cores[c]
        fc = feats[lo:hi]
        if fc.shape[0] == 0:
            fc = np.zeros((1, 32), dtype=np.float32)
        g8 = src8.shape[0]
        g4 = src4.shape[0]
        pts = np.zeros((4, cw), dtype=np.int64)
        pts[maps["rb8"][:g8], maps["col8"][:g8]] = src8
        if g4:
            pts[maps["rb4"][:g4], maps["col4"][:g4]] = src4
        feats_cm = np.empty((128, cw), dtype=np.float32)
        for rb in range(4):
            feats_cm[32 * rb:32 * (rb + 1)] = fc[pts[rb]].T
        in_maps.append({"feats": feats_cm.astype(BF16), "wcm": wcm})

    _debug_state["nc"] = nc
    _debug_state["in_maps"] = in_maps
    res = run_bass_kernel_spmd(nc, in_maps, core_ids=list(range(B)))

    out_full = np.zeros((B * NPIL, C_OUT), dtype=np.float32)
    for c in range(B):
        lo, hi, src8, pil8, src4, pil4, counts = cores[c]
        resr = np.asarray(res.results[c]["outr"]).astype(np.float32)
        resT = resr.T
        g8 = src8.shape[0]
        g4 = src4.shape[0]
        rows = np.empty((g8 + g4, C_OUT), dtype=np.float32)
        oc8 = maps["oc8"][:g8]
        l8 = maps["l8"][:g8]
        for lv in (0, 1):
            msk = l8 == lv
            acc = resT[oc8[msk, 0], 64 * lv:64 * lv + 64]
            for mm in range(1, 4):
                acc = np.maximum(acc, resT[oc8[msk, mm],
                                           64 * lv:64 * lv + 64])
            rows[:g8][msk] = acc
        if g4:
            oc4 = maps["oc4"][:g4]
            l4 = maps["l4"][:g4]
            for lv in (0, 1):
                msk = l4 == lv
                acc = np.maximum(resT[oc4[msk, 0], 64 * lv:64 * lv + 64],
                                 resT[oc4[msk, 1], 64 * lv:64 * lv + 64])
                rows[g8:][msk] = acc
        # per-pillar combine
        allpil = np.concatenate([pil8, pil4])
        order = np.argsort(allpil, kind="stable")
        sp = allpil[order]
        sr = rows[order]
        runs = np.flatnonzero(np.diff(sp, prepend=-1))
        red = np.maximum.reduceat(sr, runs, axis=0)
        upil = sp[runs]
        outc = np.zeros((NPIL, C_OUT), dtype=np.float32)
        outc[upil] = np.maximum(red + b2[None, :], np.float32(0.0))
        outc[counts == 0] = 0.0
        out_full[c * NPIL:(c + 1) * NPIL] = outc
    return out_full
